# revision 38
# baseline (speedup 1.0000x reference)
"""Trainium2 Bass kernel for T5-style relative-position-bias attention.

Problem (hardcoded): B=2, N=2048, H=16, D=64, MODEL=1024
  sim  = q @ k^T per head; sim = (sim + rel_pos_bias) * D**-0.5
  attn = softmax(sim, axis=-1)
  out  = (attn @ v) reshaped to [b, n, MODEL] @ w_out.T + b_out

Sharding: 8 cores = (batch b) x (query-chunk qoff in {0,512,1024,1536}).
Each core computes the full output rows for its 512 queries; no collectives.

Device algorithm per core (transposes all pushed to host):
  S^T[k, q] = kT_h-slices.T @ qT_h   (bf16 matmuls, contraction d=64)
  The bias enters multiplicatively: exp(C*(S+bias)) = exp(C*S)*exp(C*bias).
  T5 buckets saturate for |k-q| >= 128, so chunks of 128 keys that lie
  entirely off the diagonal band have a CONSTANT bias factor per head;
  that constant is folded into host-pre-scaled V' tiles (linearity of PV).
  Key chunks are host-permuted into a fixed slot order (const slots 0..9,
  band slots 10..15) so all 8 cores run one identical program.

  Const chunks:  P = exp(C*S^T) on the ACT engine (exact table exp).
  Band chunks:   P ~= Schraudolph bf16-bit exp on the DVE:
       bits_i16 = round(C*A*S + bterm[k,q]),  A = 128/ln2,
       bterm    = round(A*C*bias + 16256 - 7.36)  (int16, host-built),
       P~ = bitcast(bits_i16 -> bf16)  [rel std ~1.8%, zero-mean].
     One scalar_tensor_tensor per tile replaces exp AND the bias multiply,
     moving ~37% of the exp work off the saturated ACT engine.
  No max-subtraction: logits are ~N(0,1) after scaling (safe in fp32).
  O^T[m, q] = sum_k V'[k, m] * P[k, q], V' = [V | ones] (row 64 = denom r).
  Normalize: rinv = approx-recip of PSUM row 64 -> gpsimd partition
  broadcast (64 channels) -> one DVE multiply. Then y^T = w_out @ O^T +
  b_out (host re-transposes the per-core [1024, 512] outputs).

Schedule notes (why the emission looks the way it does):
  - kt is host-permuted into TILE order per hp so hp0's first transfers
    are small and demand-ordered across three DGE queues (scalar /
    gpsimd / sync); the first QK gates on ~64 KB, not a bulk transfer.
  - Dummy matmuls on a memset scratch tile pre-warm the PE HAM clock
    gate during the input-DMA window (cold MMs run at 1.2 GHz).
  - Tiles are emitted in GROUPS of 2 (2 QK pairs, then 4 PV matmuls
    from the delay queue): halving QK<->PV adjacency boundaries cuts
    un-hidden LDWEIGHTS stalls (~120ns each) on the PE.
  - The first two output-linear chains pre-accumulate mc=0..6 during
    hp7's normalize window (real work instead of scratch matmuls), and
    outputs are written bf16, two chains per DMA ([128,1024] tiles,
    2KB-per-partition descriptors) to halve writeback descriptor count.
"""
import sys
import math

sys.path.insert(0, "/opt/trn_rl_repo")

import numpy as np
import ml_dtypes

import concourse.bass as bass
from concourse import bacc
import concourse.tile as tile
from concourse import mybir
import concourse.bass_utils as _bass_utils
from concourse.bass_utils import run_bass_kernel_spmd

import os
if os.environ.get("LDW_OPT"):
    _orig_run_command = _bass_utils.run_command

    def _run_command_ldw(cmd, *a, **kw):
        cmd = ["--enable-ldw-opt=true" if c == "--enable-ldw-opt=false" else c
               for c in cmd]
        return _orig_run_command(cmd, *a, **kw)

    _bass_utils.run_command = _run_command_ldw

F32 = mybir.dt.float32
BF16 = mybir.dt.bfloat16
I16 = mybir.dt.int16

B, N, H, D = 2, 2048, 16, 64
MODEL = H * D
NQ = 512
NCORES = 8
C = float(D) ** -0.5
NUM_BUCKETS, MAX_DIST = 32, 128
CHUNKS = 16
NBAND = 6                       # band slots 10..15
NBU = 2 * NBAND                 # band units (slot, h01) per hp
AEXP = 128.0 / math.log(2.0)    # Schraudolph bf16 scale
BEXP = 16256.0 - 7.36           # zero-mean offset (tuned)

_CACHE = {}


def _slot_map(qoff):
    """Permutation slot -> original key-chunk j. Band chunks (those touching
    |k-q| < 128 for q in [qoff, qoff+512)) go to slots 10..15; if fewer than
    6 band chunks exist, nearest const chunks fill the extra band slots."""
    j_lo = max(0, -(-(qoff - 254) // 128))          # ceil((qoff-254)/128)
    j_hi = min(CHUNKS - 1, (qoff + 638) // 128)     # floor
    band = list(range(j_lo, j_hi + 1))
    while len(band) < NBAND:                        # pad with neighbors
        if band[0] > 0:
            band.insert(0, band[0] - 1)
        else:
            band.append(band[-1] + 1)
    const = [j for j in range(CHUNKS) if j not in band]
    assert len(band) == NBAND and len(const) == CHUNKS - NBAND
    return const + band                             # slots 0..9 const, 10..15 band


def _build_bass():
    nc = bacc.Bacc("TRN2", target_bir_lowering=False, debug=False,
                   num_devices=NCORES)
    qt_d = nc.dram_tensor("qt", [8, 128, NQ], BF16, kind="ExternalInput")
    kt_d = nc.dram_tensor("kt", [8, 128, N], BF16, kind="ExternalInput")
    vv_d = nc.dram_tensor("vv", [H, 128, CHUNKS, D + 1], BF16, kind="ExternalInput")
    bt_d = nc.dram_tensor("bt", [8, 128, NBU * NQ], I16, kind="ExternalInput")
    wt_d = nc.dram_tensor("wt", [8, 128, MODEL], BF16, kind="ExternalInput")
    bv_d = nc.dram_tensor("bv", [128, 8], F32, kind="ExternalInput")
    yt_d = nc.dram_tensor("yt", [4, 128, 2 * NQ], BF16, kind="ExternalOutput")

    NCONST_U = 2 * (CHUNKS - NBAND)   # units 0..19 use ACT exp

    with tile.TileContext(nc) as tc:
        with tc.tile_pool(name="const", bufs=1) as cpool:
            qt_ts = []
            for hp in range(8):
                t = cpool.tile([128, NQ], BF16, tag=f"qt{hp}", name=f"qt{hp}")
                qt_ts.append(t)
            nc.sync.dma_start(qt_ts[0], qt_d[0])
            wt_ts = []
            bv_t = None
            ocat_ts = [cpool.tile([128, NQ], BF16, tag=f"ocat{mc}",
                                  name=f"ocat{mc}")
                       for mc in range(8)]

            with tc.tile_pool(name="kt", bufs=2) as ktpool, \
                 tc.tile_pool(name="vv", bufs=4) as vvpool, \
                 tc.tile_pool(name="bt", bufs=2) as btpool, \
                 tc.tile_pool(name="p0", bufs=8) as p0pool, \
                 tc.tile_pool(name="pm", bufs=8) as pmpool, \
                 tc.tile_pool(name="sm", bufs=6) as smpool, \
                 tc.tile_pool(name="stps", bufs=2, space="PSUM") as stp, \
                 tc.tile_pool(name="ops", bufs=4, space="PSUM") as opool:
                # Flat software-pipelined stream over all pairs' tiles.
                # Unit = (slot, h01); units come in slot-pairs so QK matmuls
                # of h0 (PE rows 0-63) and h1 (rows 64-127) are adjacent and
                # run concurrently in disjoint row groups. Tiles = 2 units.
                # Const pairs (ACT exp) and DVE pairs (slots 8/9 imm
                # Schraudolph, band slots 10..15 STT Schraudolph) are
                # interleaved so ACT and DVE stream concurrently. Tiles are
                # emitted in GROUPS of 2 (QK A, QK B, then 2 tiles' PVs from
                # the delay queue): halving the QK<->PV adjacency boundaries
                # cuts un-hidden LDWEIGHTS stalls on the PE. PV matmuls are
                # emitted DELAY tiles late so later QKs bridge boundaries.
                DELAY = 4
                # Each hp OPENS with two ACT tiles: the previous hp's
                # normalize chain (recip + 2 multiplies on DVE) lands in that
                # window, so band exps don't stall behind it.
                PAIR_ORDER = [0, 9, 1, 10, 2, 11, 3, 12, 4, 13,
                              5, 14, 6, 15, 7, 8]
                # hp7 ends on ACT tiles so its normalize chain (which gates
                # the output linear) starts as early as possible
                PAIR_ORDER7 = [0, 9, 1, 10, 2, 11, 3, 12, 4, 13,
                               5, 14, 6, 15, 8, 7]
                TRIG = 4        # tile index at which hp+1's inputs prefetch
                tiles = []
                for hp in range(8):
                    po = PAIR_ORDER7 if hp == 7 else PAIR_ORDER
                    for i, s in enumerate(po):
                        tiles.append((hp, i, [(s, 0), (s, 1)],
                                      i == 0, i == len(po) - 1))
                state = {}      # hp -> (kt_t, vv_ts, bt_t)
                opss = {}       # hp -> [o_ps h0, o_ps h1]
                pv_queue = []   # (hp, chunk, srcs, first, last)

                def setup_hp(hp):
                    """SBUF input tiles + DMAs for head-pair hp (prefetchable).
                    kt is host-permuted into TILE order per hp (tile i's
                    stationary = cols [i*128:(i+1)*128]), so hp0's transfers
                    can be demand-ordered: the first QK gates on 64 KB, not
                    on a bulk slot-ordered transfer."""
                    kt_t = ktpool.tile([128, N], BF16, tag="kt", name="kt")
                    bt_t = btpool.tile([128, NBU * NQ], I16,
                                       tag="bt", name="bt")
                    if hp == 0:
                        # demand-ordered start. scalar+sync queues wake
                        # first and carry everything deadline-critical;
                        # gpsimd (whose init ends late) gets only the
                        # far-band bterms, whose exps run last.
                        nc.scalar.dma_start(kt_t[:, :256], kt_d[hp][:, :256])
                        vv0 = vvpool.tile([128, CHUNKS, D + 1], BF16,
                                          tag="vv", name="vv0")
                        nc.scalar.dma_start(vv0, vv_d[0])
                        nc.sync.dma_start(kt_t[:, 256:768],
                                          kt_d[hp][:, 256:768])
                        nc.sync.dma_start(bt_t[:, :2 * NQ],
                                          bt_d[hp][:, :2 * NQ])
                        vv1 = vvpool.tile([128, CHUNKS, D + 1], BF16,
                                          tag="vv", name="vv1")
                        nc.sync.dma_start(vv1, vv_d[1])
                        state["vv0"] = [vv0, vv1]
                        nc.sync.dma_start(bt_t[:, 2 * NQ:4 * NQ],
                                          bt_d[hp][:, 2 * NQ:4 * NQ])
                        nc.sync.dma_start(kt_t[:, 768:1536],
                                          kt_d[hp][:, 768:1536])
                        nc.sync.dma_start(bt_t[:, 4 * NQ:6 * NQ],
                                          bt_d[hp][:, 4 * NQ:6 * NQ])
                        nc.sync.dma_start(kt_t[:, 1536:], kt_d[hp][:, 1536:])
                        nc.gpsimd.dma_start(bt_t[:, 6 * NQ:],
                                            bt_d[hp][:, 6 * NQ:])
                    else:
                        nc.sync.dma_start(kt_t, kt_d[hp])
                        nc.sync.dma_start(bt_t, bt_d[hp])
                    if hp == 0:
                        vv_ts = state.pop("vv0")
                    else:
                        vv_ts = []
                        for h01 in range(2):
                            vv_t = vvpool.tile([128, CHUNKS, D + 1], BF16,
                                               tag="vv", name="vv")
                            nc.sync.dma_start(vv_t, vv_d[2 * hp + h01])
                            vv_ts.append(vv_t)
                    state[hp] = (kt_t, vv_ts, bt_t)

                def emit_pv(hp, u0, chunk, srcs):
                    vv_ts = state[hp][1]
                    o_pss = opss[hp]
                    for uu, (s, h01) in enumerate(chunk):
                        g = u0 + uu
                        nc.tensor.matmul(
                            o_pss[h01], vv_ts[h01][:, s, :], srcs[uu],
                            start=(g < 2), stop=(g >= 30))

                def emit_normalize(hp):
                    # both recips issue before the first broadcast-dependent
                    # multiply so the DVE FIFO never blocks on gpsimd
                    o_pss = opss[hp]
                    rbs = {}
                    for h01 in (1, 0):
                        o_ps = o_pss[h01]
                        # PSUM->SBUF stage on DVE (ACT is exp-saturated at
                        # hp boundaries; a copy there stalls the QK pipeline)
                        rstage = smpool.tile([1, NQ], F32, tag="rstage",
                                             name="rstage")
                        nc.vector.tensor_scalar_mul(rstage, o_ps[64:65, :],
                                                    1.0)
                        rinv = smpool.tile([1, NQ], F32, tag="rinv",
                                           name="rinv")
                        nc.vector.reciprocal_approx_fast(rinv, rstage)
                        # broadcast only the 64 partitions actually used
                        rb = smpool.tile([64, NQ], F32, tag="rb", name="rb")
                        nc.gpsimd.partition_broadcast(rb, rinv)
                        rbs[h01] = rb
                    for h01 in (1, 0):
                        o_ps, rb = o_pss[h01], rbs[h01]
                        if h01 == 0:
                            nc.vector.tensor_tensor(
                                ocat_ts[hp][0:64, :], o_ps[0:64, :],
                                rb[0:64, :], mybir.AluOpType.mult)
                        else:
                            s64 = smpool.tile([64, NQ], BF16, tag="s64",
                                              name="s64")
                            nc.vector.tensor_tensor(
                                s64, o_ps[0:64, :], rb[0:64, :],
                                mybir.AluOpType.mult)
                            nc.gpsimd.dma_start(ocat_ts[hp][64:128, :], s64)

                # PE pre-warm: the HAM clock gate holds the PE at 1.2 GHz
                # until ~3.4us of sustained activity. The first real matmul
                # can't start until the input DMAs land (~9.6us in); dummy
                # matmuls on a memset scratch tile during that window ramp
                # the clock so real tiles start at 2.4 GHz.
                warm_sb = cpool.tile([128, 128], BF16, tag="warm",
                                     name="warm")
                nc.vector.memset(warm_sb, 0.0)
                warm_ps = opool.tile([128, 128], F32, tag="ops",
                                     name="warm_ps")
                for _ in range(16):
                    nc.tensor.matmul(warm_ps, warm_sb, warm_sb,
                                     start=True, stop=True,
                                     skip_group_check=True)

                setup_hp(0)
                for hp, ti, chunk, is_first, is_last in tiles:
                    if is_first:
                        if hp + 1 < 8:
                            nc.sync.dma_start(qt_ts[hp + 1], qt_d[hp + 1])
                        if hp == 0:
                            bv_t = cpool.tile([128, 8], F32, tag="bv",
                                              name="bv")
                            nc.sync.dma_start(bv_t, bv_d[:, :])
                        opss[hp] = [opool.tile([D + 1, NQ], F32,
                                               tag="ops", name="ops")
                                    for _ in range(2)]
                    if ti == 2:
                        # one w_out chunk per hp, spread so the transfers
                        # never collide with the kt/vv/bt prefetches
                        t = cpool.tile([128, MODEL], BF16,
                                       tag=f"wt{hp}", name=f"wt{hp}")
                        nc.sync.dma_start(t, wt_d[hp])
                        wt_ts.append(t)
                    if ti == TRIG and hp + 1 < 8:
                        setup_hp(hp + 1)
                    kt_t, vv_ts, bt_t = state[hp]
                    nu = len(chunk)
                    st = stp.tile([128, 2 * NQ], F32, tag="st", name="st")
                    for uu, (s, h01) in enumerate(chunk):
                        lo, hi = h01 * 64, h01 * 64 + 64
                        # kt columns are host-ordered by TILE index, not slot
                        nc.tensor.matmul(
                            st[:, uu * NQ:(uu + 1) * NQ],
                            kt_t[lo:hi, ti * 128:(ti + 1) * 128],
                            qt_ts[hp][lo:hi, :],
                            start=True, stop=True)
                    # contiguous engine runs within the tile
                    srcs = [None] * nu
                    runs = []
                    for uu, (s, h01) in enumerate(chunk):
                        if s < CHUNKS - NBAND - 1:
                            eng = "act"
                        elif s < CHUNKS - NBAND:
                            eng = "dve9"
                        else:
                            eng = "dve"
                        if runs and runs[-1][0] == eng and (
                                eng == "act" or chunk[runs[-1][1]][0] == s):
                            runs[-1][2] += 1
                        else:
                            runs.append([eng, uu, 1])
                    for eng, uu0, ln in runs:
                        cols = slice(uu0 * NQ, (uu0 + ln) * NQ)
                        if eng == "act":
                            p0 = p0pool.tile([128, 2 * NQ], BF16, tag="p0",
                                             name="p0")
                            nc.scalar.activation(
                                p0[:, :ln * NQ], st[:, cols],
                                mybir.ActivationFunctionType.Exp,
                                bias=0.0, scale=C)
                            for k in range(ln):
                                srcs[uu0 + k] = p0[:, k * NQ:(k + 1) * NQ]
                        else:
                            s0, h010 = chunk[uu0]
                            pm = pmpool.tile([128, 2 * NQ], I16, tag="pm",
                                             name="pm")
                            if eng == "dve9":        # slot 9: imm Schraudolph
                                nc.vector.tensor_scalar(
                                    pm[:, :ln * NQ], st[:, cols],
                                    float(C * AEXP), float(BEXP),
                                    mybir.AluOpType.mult,
                                    mybir.AluOpType.add)
                            else:                    # band: bterm Schraudolph
                                j0 = 2 * (s0 - (CHUNKS - NBAND)) + h010
                                nc.vector.scalar_tensor_tensor(
                                    pm[:, :ln * NQ], st[:, cols],
                                    float(C * AEXP),
                                    bt_t[:, j0 * NQ:(j0 + ln) * NQ],
                                    mybir.AluOpType.mult,
                                    mybir.AluOpType.add)
                            for k in range(ln):
                                srcs[uu0 + k] = pm[
                                    :, k * NQ:(k + 1) * NQ].bitcast(BF16)
                    pv_queue.append((hp, 2 * ti, chunk, srcs, is_last))
                    if ti % 2 == 1:
                        # defer 2 tiles' PVs across each hp boundary: the
                        # next hp's first groups then have extra PV cover
                        # while its first ACT exps recycle the st slots
                        target = DELAY + (2 if (ti == 15 and hp < 7) else 0)
                        while len(pv_queue) > target:
                            qhp, qu0, qchunk, qsrcs, qlast = pv_queue.pop(0)
                            emit_pv(qhp, qu0, qchunk, qsrcs)
                            if qlast:
                                emit_normalize(qhp)
                for qhp, qu0, qchunk, qsrcs, qlast in pv_queue:
                    emit_pv(qhp, qu0, qchunk, qsrcs)
                    if qlast:
                        emit_normalize(qhp)
                # Bridge hp7's normalize/ocat-DMA window (idle >3.4us here
                # would drop the p-state clock) with REAL work: the first two
                # output-linear chains' mc=0..6 partial accumulations don't
                # need hp7's ocat and fit exactly in the gap. Their PSUM
                # comes from the ops pool (hp6's slots are free by now).
                pre_fps = []
                for oc in range(2):
                    fp = opool.tile([128, NQ], F32, tag="ops",
                                    name=f"pfp{oc}")
                    for mc in range(7):
                        nc.tensor.matmul(
                            fp, wt_ts[mc][:, oc * 128:(oc + 1) * 128],
                            ocat_ts[mc], start=(mc == 0), stop=False)
                    pre_fps.append(fp)
                ysbp = cpool.tile([128, 2 * NQ], BF16, tag="ysbp",
                                  name="ysbp")
                for oc in range(2):
                    fp = pre_fps[oc]
                    nc.tensor.matmul(
                        fp, wt_ts[7][:, oc * 128:(oc + 1) * 128],
                        ocat_ts[7], start=False, stop=True)
                    nc.scalar.add(ysbp[:, oc * NQ:(oc + 1) * NQ], fp,
                                  bv_t[:, oc:oc + 1])
                # one DMA per output pair: 2KB-per-partition descriptors,
                # half the descriptor count of per-oc transfers
                nc.scalar.dma_start(yt_d[0], ysbp)

            with tc.tile_pool(name="ysb", bufs=2) as ypool, \
                 tc.tile_pool(name="fin", bufs=4, space="PSUM") as fpool:
                # remaining 6 single-oc chains, paired into [128, 1024]
                # output tiles (one 2KB-descriptor DMA per pair)
                for pair in range(1, 4):
                    ysb = ypool.tile([128, 2 * NQ], BF16, tag="ysb",
                                     name="ysb")
                    for sub in range(2):
                        oc = 2 * pair + sub
                        fp = fpool.tile([128, NQ], F32, tag="fp", name="fp")
                        for mc in range(8):
                            nc.tensor.matmul(
                                fp, wt_ts[mc][:, oc * 128:(oc + 1) * 128],
                                ocat_ts[mc], start=(mc == 0), stop=(mc == 7))
                        if oc < 7:
                            # ACT add: the DMA triggers from the same queue
                            nc.scalar.add(ysb[:, sub * NQ:(sub + 1) * NQ],
                                          fp, bv_t[:, oc:oc + 1])
                        else:
                            # last chain: DVE add (faster) shortens the tail
                            nc.vector.tensor_scalar_add(
                                ysb[:, sub * NQ:(sub + 1) * NQ], fp,
                                bv_t[:, oc:oc + 1])
                    nc.scalar.dma_start(yt_d[pair], ysb)
    nc.compile()
    return nc


def _rel_pos_bucket_np(rel):
    """T5 bidirectional bucketing, float32 math mirroring the jnp reference."""
    nb = NUM_BUCKETS // 2
    ret = (rel >= 0).astype(np.int32) * nb
    n = np.abs(rel)
    max_exact = nb // 2
    is_small = n < max_exact
    n_safe = np.maximum(n, 1).astype(np.float32)
    val_large = max_exact + (
        np.log(n_safe / np.float32(max_exact)).astype(np.float32)
        / np.float32(math.log(MAX_DIST / max_exact)) * np.float32(nb - max_exact)
    ).astype(np.int32)
    val_large = np.minimum(val_large, nb - 1)
    return ret + np.where(is_small, n, val_large)


def _l_diag(rel_emb):
    """ldiag[h, r + 2047] = C * rel_emb[bucket(r), h], r in [-2047, 2047]."""
    rel = np.arange(-2047, 2048, dtype=np.int32)
    buckets = _rel_pos_bucket_np(rel)
    e = np.float32(C) * np.asarray(rel_emb, np.float32)[buckets, :]
    return np.ascontiguousarray(e.T)                 # [H, 4095]


def _prep_inputs(q, k, v, rel_emb, w_out, b_out):
    q = np.asarray(q, np.float32)
    k = np.asarray(k, np.float32)
    v = np.asarray(v, np.float32)
    rel_emb = np.asarray(rel_emb, np.float32)
    ldiag = _l_diag(rel_emb)
    # constant bias factors of the two saturated bucket regions, per head
    e_pos = np.exp(np.float32(C) * rel_emb[31, :])   # k - q >= 128
    e_neg = np.exp(np.float32(C) * rel_emb[15, :])   # k - q <= -128
    wt = np.ascontiguousarray(np.asarray(w_out, np.float32).T).reshape(8, 128, MODEL)
    bv = np.ascontiguousarray(np.asarray(b_out, np.float32).reshape(8, 128).T)
    p = np.arange(128)
    u = np.arange(NQ)
    # kernel tile order per head-pair (must match PAIR_ORDER/PAIR_ORDER7)
    PO = [0, 9, 1, 10, 2, 11, 3, 12, 4, 13, 5, 14, 6, 15, 7, 8]
    PO7 = [0, 9, 1, 10, 2, 11, 3, 12, 4, 13, 5, 14, 6, 15, 8, 7]
    in_maps = []
    for core in range(NCORES):
        b, qc = divmod(core, 4)
        qoff = qc * NQ
        smap = _slot_map(qoff)                       # slot -> chunk j
        qs = q[b, qoff:qoff + NQ].reshape(NQ, 8, 2, 64)
        qt = np.ascontiguousarray(qs.transpose(1, 2, 3, 0)).reshape(8, 128, NQ)
        kt = np.ascontiguousarray(
            k[b].reshape(N, 8, 2, 64).transpose(1, 2, 3, 0)).reshape(8, 128, N)
        # slot order, then per-hp TILE order (kernel reads cols by tile idx)
        kt = kt.reshape(8, 128, CHUNKS, 128)[:, :, smap, :]
        kt = np.stack([kt[hp][:, PO7 if hp == 7 else PO, :]
                       for hp in range(8)])
        kt = np.ascontiguousarray(kt).reshape(8, 128, N)
        vs = v[b].reshape(CHUNKS, 128, H, D).transpose(2, 1, 0, 3)  # [h,kk,j,d]
        vv = np.concatenate(
            [vs, np.ones((H, 128, CHUNKS, 1), np.float32)], axis=-1)
        vv = vv[:, :, smap, :]                       # slot order
        # scale const slots by their constant bias factor; band slots (>=10)
        # get the full bias via the Schraudolph bterm instead, even if
        # saturated
        for s in range(CHUNKS - NBAND):
            j = smap[s]
            rel_min = 128 * j - qoff - (NQ - 1)      # min over tile of k - q
            rel_max = 128 * j + 127 - qoff
            if rel_min >= 128:
                fac = e_pos
            elif rel_max <= -128:
                fac = e_neg
            else:
                raise AssertionError(
                    f"band chunk {j} in const slot {s} (qoff={qoff})")
            vv[:, :, s, :] *= fac[:, None, None]
        # bterm[hp][p, (sb, h01), u] int16: round(A * C*bias + BEXP)
        bt = np.empty((8, 128, NBAND, 2, NQ), np.float32)
        for sb in range(NBAND):
            j = smap[10 + sb]
            idx = (128 * j + p[:, None]) - (qoff + u[None, :]) + 2047
            lb = ldiag[:, idx]                       # [H, 128, NQ]
            bt[:, :, sb, 0, :] = lb[0::2]
            bt[:, :, sb, 1, :] = lb[1::2]
        bt_i = np.rint(np.float32(AEXP) * bt + np.float32(BEXP)).astype(np.int16)
        in_maps.append({
            "qt": qt.astype(ml_dtypes.bfloat16),
            "kt": kt.astype(ml_dtypes.bfloat16),
            "vv": np.ascontiguousarray(vv).astype(ml_dtypes.bfloat16),
            "bt": np.ascontiguousarray(bt_i.reshape(8, 128, NBU * NQ)),
            "wt": wt.astype(ml_dtypes.bfloat16), "bv": bv,
        })
    return in_maps


def _run(q, k, v, rel_emb, w_out, b_out, trace=False):
    if "nc" not in _CACHE:
        _CACHE["nc"] = _build_bass()
    nc = _CACHE["nc"]
    in_maps = _prep_inputs(q, k, v, rel_emb, w_out, b_out)
    res = run_bass_kernel_spmd(nc, in_maps, core_ids=list(range(NCORES)),
                               trace=trace)
    y = np.empty((B, N, MODEL), np.float32)
    for core in range(NCORES):
        b, qc = divmod(core, 4)
        qoff = qc * NQ
        yt = np.asarray(res.results[core]["yt"], dtype=np.float32)
        y[b, qoff:qoff + NQ] = (yt.reshape(4, 128, 2, NQ)
                                .transpose(3, 0, 2, 1).reshape(NQ, MODEL))
    return y, res


def kernel(q, k, v, rel_emb, w_out, b_out):
    y, _ = _run(q, k, v, rel_emb, w_out, b_out, trace=False)
    return y

